# revision 9
# baseline (speedup 1.0000x reference)
"""AttentionPooling (segment softmax-pool) Trainium2 kernel, 8-core SPMD.

Math (faithful to the reference up to O(s^2), s = global-softmax values <= 6.4e-5):
  l_i = x_i . W + b;  E_i = exp(l_i);  Z = sum_i E_i  (global, one AllReduce)
  s_i = E_i / Z
  per-segment softmax of s with max-subtraction cancels exactly:
      a_i = exp(s_i) / sum_{j in g} exp(s_j)
  first-order Taylor (exp(s) = 1 + s, relative error ~ s^2/2 ~ 2e-9):
      out_g = (M0_g + M1_g / Z) / (n_g + S_g / Z)
  with per-segment sums  M0 = sum x_i,  M1 = sum E_i x_i,  S = sum E_i,
  n_g = node count.  All segment sums are core-local (segments are sharded
  by contiguous sorted batch-id ranges); only Z needs the AllReduce.

Precision: x is split on the host into fp16 hi + fp16 lo (hi+lo ~ 22-bit
mantissa).  M0 accumulates both halves into the same PSUM bank in fp32, so
M0 is fp32-accurate while every matmul runs at full (1 cycle/row) PE rate.
M1 and S are ~3e-5-relative corrections, so fp16 inputs are ample for them.

Layout per core: 512 segments = 4 phases x 128 segments (PSUM partition dim).
Each phase's nodes are padded to C chunks of 128 nodes; a [128 nodes x 128
segs] one-hot (generated on-device from relative batch ids) turns the
per-phase segment sums into PE matmuls.
"""

import math

import numpy as np

N = 262144
HIDDEN = 512
B = 4096
NCORES = 8
SEGS_PER_CORE = B // NCORES  # 512
PHASES = 4
SEGW = SEGS_PER_CORE // PHASES  # 128 segments per phase
P = 128  # partitions / chunk size
BLK = 8  # chunks per x DMA block (1 MiB fp16 per dma_start)

_program_cache = {}


def _build_program(C, variant=None):
    """Build + compile the 8-core SPMD program for C chunks per phase.

    variant flags (for HW-hang bisection):
      bcast_engine: 'sync' | 'gpsimd'   engine for broadcast/const DMAs
      pe_reduce:    True -> cross-partition Z reduce + invZ broadcast via PE
      collective:   False -> skip AllReduce (wrong Z scale, debug only)
      use_ttr:      False -> mult + tensor_reduce instead of fused TTR
    """
    v = {"bcast_engine": "gpsimd", "pe_reduce": True, "collective": True,
         "use_ttr": False}
    if variant:
        v.update(variant)
    import concourse.bacc as bacc
    import concourse.tile as tile
    from concourse import mybir

    f16 = mybir.dt.float16
    f32 = mybir.dt.float32
    Alu = mybir.AluOpType
    Act = mybir.ActivationFunctionType

    NODES = PHASES * C * P
    nc = bacc.Bacc("TRN2", target_bir_lowering=False, debug=False,
                   num_devices=NCORES)

    xhi = nc.dram_tensor("xhi", [NODES, HIDDEN], f16, kind="ExternalInput").ap()
    xlo = nc.dram_tensor("xlo", [NODES, HIDDEN], f16, kind="ExternalInput").ap()
    rel = nc.dram_tensor("rel", [PHASES, P, C], f32, kind="ExternalInput").ap()
    cnts = nc.dram_tensor("cnts", [PHASES, P, 1], f32, kind="ExternalInput").ap()
    wrow = nc.dram_tensor("wrow", [1, HIDDEN], f16, kind="ExternalInput").ap()
    brow = nc.dram_tensor("brow", [1, 1], f32, kind="ExternalInput").ap()
    irow = nc.dram_tensor("irow", [1, P], f16, kind="ExternalInput").ap()
    outp = nc.dram_tensor("out", [SEGS_PER_CORE, HIDDEN], f32,
                          kind="ExternalOutput").ap()

    NB = math.ceil(C / BLK)

    with tile.TileContext(nc) as tc:
        with (
            tc.tile_pool(name="singles", bufs=1) as singles,
            tc.tile_pool(name="hi", bufs=3) as hipool,
            tc.tile_pool(name="lo", bufs=3) as lopool,
            tc.tile_pool(name="relp", bufs=2) as relpool,
            tc.tile_pool(name="oh", bufs=4) as ohpool,
            tc.tile_pool(name="dump", bufs=2) as dumppool,
            tc.tile_pool(name="small", bufs=6) as smallpool,
            tc.tile_pool(name="xe", bufs=3) as xepool,
            tc.tile_pool(name="outb", bufs=2) as outpool,
            tc.tile_pool(name="pm0", bufs=2, space="PSUM") as pm0,
            tc.tile_pool(name="pm1", bufs=2, space="PSUM") as pm1,
            tc.tile_pool(name="pms", bufs=2, space="PSUM") as pms,
            tc.tile_pool(name="pep", bufs=1, space="PSUM") as pep,
            tc.tile_pool(name="dram", bufs=1, space="DRAM") as drampool,
        ):
            # ---- constants (broadcast along partitions) ----
            bce = nc.sync if v["bcast_engine"] == "sync" else nc.gpsimd
            Wb = singles.tile([P, HIDDEN], f16)
            bce.dma_start(out=Wb[:], in_=wrow.to_broadcast([P, HIDDEN]))
            bb = singles.tile([P, 1], f32)
            bce.dma_start(out=bb[:], in_=brow.to_broadcast([P, 1]))
            iob = singles.tile([P, P], f16)
            bce.dma_start(out=iob[:], in_=irow.to_broadcast([P, P]))
            cnt_t = singles.tile([P, PHASES], f32)
            for p in range(PHASES):
                bce.dma_start(out=cnt_t[:, p:p + 1], in_=cnts[p])
            if v["pe_reduce"]:
                ones128 = singles.tile([P, 1], f32)
                nc.vector.memset(ones128[:], 1.0)

            maccs = {}
            ssml = {}

            for p in range(PHASES):
                rel_t = relpool.tile([P, C], f32)
                nc.sync.dma_start(out=rel_t[:], in_=rel[p])

                m0 = pm0.tile([P, HIDDEN], f32)
                m1 = pm1.tile([P, HIDDEN], f32)
                ms = pms.tile([P, 1], f32)

                for bi in range(NB):
                    c0 = bi * BLK
                    nb = min(BLK, C - c0)
                    r0 = (p * C + c0) * P
                    hi_t = hipool.tile([P, BLK, HIDDEN], f16)
                    lo_t = lopool.tile([P, BLK, HIDDEN], f16)
                    src_hi = xhi[r0:r0 + nb * P, :].rearrange(
                        "(c q) h -> q c h", q=P)
                    src_lo = xlo[r0:r0 + nb * P, :].rearrange(
                        "(c q) h -> q c h", q=P)
                    nc.sync.dma_start(out=hi_t[:, :nb, :], in_=src_hi)
                    nc.sync.dma_start(out=lo_t[:, :nb, :], in_=src_lo)

                    for ci in range(nb):
                        c = c0 + ci
                        hi_c = hi_t[:, ci, :]
                        lo_c = lo_t[:, ci, :]

                        # one-hot [node, seg-in-phase] from relative batch id
                        oh = ohpool.tile([P, P], f16)
                        nc.vector.tensor_scalar(
                            out=oh[:], in0=iob[:], scalar1=rel_t[:, c:c + 1],
                            scalar2=None, op0=Alu.is_equal)

                        # logits: l = sum_h hi*W.  DVE does the elementwise
                        # product; ScalarE's activation-accumulate does the
                        # free-dim sum in fp32.  (The fused DVE
                        # tensor_tensor_reduce crashes on HW via this path.)
                        dump = dumppool.tile([P, HIDDEN], f16)
                        lt = smallpool.tile([P, 1], f32, tag="lt")
                        if v["use_ttr"]:
                            nc.vector.tensor_tensor_reduce(
                                out=dump[:], in0=hi_c, in1=Wb[:], scale=1.0,
                                scalar=0.0, op0=Alu.mult, op1=Alu.add,
                                accum_out=lt[:])
                        else:
                            nc.vector.tensor_mul(out=dump[:], in0=hi_c,
                                                 in1=Wb[:])
                            dump2 = dumppool.tile([P, HIDDEN], f16,
                                                  tag="dump2")
                            nc.scalar.activation(out=dump2[:], in_=dump[:],
                                                 func=Act.Copy, scale=1.0,
                                                 accum_out=lt[:])

                        # E = exp(l + b); f32 for the tensor_scalar scalar,
                        # fp16 copy for the PE rhs (ample: feeds O(s) terms)
                        ef = smallpool.tile([P, 1], f32, tag="ef")
                        nc.scalar.activation(out=ef[:], in_=lt[:], func=Act.Exp,
                                             bias=bb[:], scale=1.0)
                        eh = smallpool.tile([P, 1], f16, tag="eh")
                        nc.vector.tensor_copy(eh[:], ef[:])

                        # xe = E * hi
                        xe = xepool.tile([P, HIDDEN], f16)
                        nc.vector.tensor_scalar_mul(out=xe[:], in0=hi_c,
                                                    scalar1=ef[:])

                        first = c == 0
                        last = c == C - 1
                        nc.tensor.matmul(m0[:], oh[:], hi_c, start=first,
                                         stop=False)
                        nc.tensor.matmul(m0[:], oh[:], lo_c, start=False,
                                         stop=last)
                        nc.tensor.matmul(m1[:], oh[:], xe[:], start=first,
                                         stop=last)
                        nc.tensor.matmul(ms[:], oh[:], eh[:], start=first,
                                         stop=last)

                # drain phase accumulators PSUM -> SBUF
                a0 = singles.tile([P, HIDDEN], f32, tag=f"macc0_{p}")
                a1 = singles.tile([P, HIDDEN], f32, tag=f"macc1_{p}")
                sv = singles.tile([P, 1], f32, tag=f"ssml_{p}")
                nc.vector.tensor_copy(a0[:], m0[:])
                nc.vector.tensor_copy(a1[:], m1[:])
                nc.vector.tensor_copy(sv[:], ms[:])
                maccs[p] = (a0, a1)
                ssml[p] = sv

            # ---- global Z via AllReduce ----
            s01 = singles.tile([P, 1], f32, tag="s01")
            s23 = singles.tile([P, 1], f32, tag="s23")
            sall = singles.tile([P, 1], f32, tag="sall")
            nc.vector.tensor_add(out=s01[:], in0=ssml[0][:], in1=ssml[1][:])
            nc.vector.tensor_add(out=s23[:], in0=ssml[2][:], in1=ssml[3][:])
            nc.vector.tensor_add(out=sall[:], in0=s01[:], in1=s23[:])
            zl = singles.tile([1, 1], f32, tag="zl")
            if v["pe_reduce"]:
                pz = pep.tile([1, 1], f32, tag="pz")
                nc.tensor.matmul(pz[:], ones128[:], sall[:], start=True,
                                 stop=True)
                nc.vector.tensor_copy(zl[:], pz[:])
            else:
                nc.gpsimd.tensor_reduce(out=zl[:], in_=sall[:],
                                        axis=mybir.AxisListType.C, op=Alu.add)
            zg = singles.tile([1, 1], f32, tag="zg")
            if v["collective"]:
                in_b = drampool.tile([1, 1], f32, tag="cc_in")
                out_b = drampool.tile([1, 1], f32, tag="cc_out")
                nc.gpsimd.dma_start(out=in_b[:], in_=zl[:])
                nc.gpsimd.collective_compute(
                    "AllReduce", Alu.add,
                    replica_groups=[list(range(NCORES))],
                    ins=[in_b.opt()], outs=[out_b.opt()])
                nc.gpsimd.dma_start(out=zg[:], in_=out_b[:])
            else:
                nc.vector.tensor_copy(zg[:], zl[:])
            izb = singles.tile([P, 1], f32, tag="izb")
            if v["pe_reduce"]:
                ones_row = singles.tile([1, P], f32)
                nc.vector.memset(ones_row[:], 1.0)
                pzb = pep.tile([P, 1], f32, tag="pzb")
                nc.tensor.matmul(pzb[:], ones_row[:], zg[:],
                                 start=True, stop=True)
                nc.vector.reciprocal(out=izb[:], in_=pzb[:])
            else:
                iz = singles.tile([1, 1], f32, tag="iz")
                nc.vector.reciprocal(out=iz[:], in_=zg[:])
                nc.gpsimd.partition_broadcast(izb[:], iz[:])

            # ---- combine: out = (M0 + M1/Z) / (n + S/Z) ----
            for p in range(PHASES):
                a0, a1 = maccs[p]
                d = smallpool.tile([P, 1], f32, tag="d")
                nc.vector.scalar_tensor_tensor(
                    out=d[:], in0=ssml[p][:], scalar=izb[:],
                    in1=cnt_t[:, p:p + 1], op0=Alu.mult, op1=Alu.add)
                r = smallpool.tile([P, 1], f32, tag="r")
                nc.vector.reciprocal(out=r[:], in_=d[:])
                t = outpool.tile([P, HIDDEN], f32, tag="t")
                nc.vector.scalar_tensor_tensor(
                    out=t[:], in0=a1[:], scalar=izb[:], in1=a0[:],
                    op0=Alu.mult, op1=Alu.add)
                o = outpool.tile([P, HIDDEN], f32, tag="o")
                nc.vector.tensor_scalar_mul(out=o[:], in0=t[:], scalar1=r[:])
                nc.sync.dma_start(out=outp[p * SEGW:(p + 1) * SEGW, :],
                                  in_=o[:])

    nc.compile()
    return nc


def _prepare(x, batch, W, b, force_C=None):
    """Host-side shard/pad/split. Returns (C, in_maps)."""
    counts = np.bincount(batch, minlength=B).astype(np.int64)
    bounds = np.zeros(B + 1, dtype=np.int64)
    np.cumsum(counts, out=bounds[1:])

    phase_n = np.zeros((NCORES, PHASES), dtype=np.int64)
    for k in range(NCORES):
        s0 = k * SEGS_PER_CORE
        for p in range(PHASES):
            phase_n[k, p] = (bounds[s0 + (p + 1) * SEGW] -
                             bounds[s0 + p * SEGW])
    C = int(math.ceil(phase_n.max() / P))
    if force_C is not None:
        assert force_C >= C
        C = force_C

    xhi = x.astype(np.float16)
    xlo = (x - xhi.astype(np.float32)).astype(np.float16)

    wrow = W[:, 0].astype(np.float16).reshape(1, HIDDEN)
    brow = np.asarray(b, dtype=np.float32).reshape(1, 1)
    irow = np.arange(P, dtype=np.float16).reshape(1, P)

    in_maps = []
    for k in range(NCORES):
        s0 = k * SEGS_PER_CORE
        xhi_k = np.zeros((PHASES * C * P, HIDDEN), dtype=np.float16)
        xlo_k = np.zeros((PHASES * C * P, HIDDEN), dtype=np.float16)
        rel_k = np.full((PHASES, P, C), -1.0, dtype=np.float32)
        cnt_k = np.zeros((PHASES, P, 1), dtype=np.float32)
        for p in range(PHASES):
            lo_i = int(bounds[s0 + p * SEGW])
            hi_i = int(bounds[s0 + (p + 1) * SEGW])
            n = hi_i - lo_i
            dst0 = p * C * P
            xhi_k[dst0:dst0 + n] = xhi[lo_i:hi_i]
            xlo_k[dst0:dst0 + n] = xlo[lo_i:hi_i]
            r = np.full(C * P, -1.0, dtype=np.float32)
            r[:n] = (batch[lo_i:hi_i] - (s0 + p * SEGW)).astype(np.float32)
            rel_k[p] = r.reshape(C, P).T
            cnt_k[p, :, 0] = counts[s0 + p * SEGW:s0 + (p + 1) * SEGW]
        in_maps.append({
            "xhi": xhi_k, "xlo": xlo_k, "rel": rel_k, "cnts": cnt_k,
            "wrow": wrow, "brow": brow, "irow": irow,
        })
    return C, in_maps


def run(inputs, trace=False, trace_kwargs=None):
    """Run the kernel; returns (out [B, HIDDEN] f32, BassKernelResults)."""
    from concourse.bass_utils import run_bass_kernel_spmd

    x = np.asarray(inputs["x"], dtype=np.float32)
    batch = np.asarray(inputs["batch"]).astype(np.int64)
    W = np.asarray(inputs["W"], dtype=np.float32)
    b = np.asarray(inputs["b"], dtype=np.float32)

    C, in_maps = _prepare(x, batch, W, b)
    if C not in _program_cache:
        _program_cache[C] = _build_program(C)
    nc = _program_cache[C]

    kwargs = {}
    if trace:
        kwargs["trace"] = True
        if trace_kwargs:
            kwargs.update(trace_kwargs)
    res = run_bass_kernel_spmd(nc, in_maps, core_ids=list(range(NCORES)),
                               **kwargs)
    out = np.concatenate([res.results[k]["out"] for k in range(NCORES)],
                         axis=0).astype(np.float32)
    return out, res


def kernel(**inputs):
    out, _ = run(inputs, trace=False)
    return out
